# revision 1
# baseline (speedup 1.0000x reference)
"""Differential attention kernel for 8 Trainium2 NeuronCores.

Problem: B=2, T=2048, D=2048, H=16 heads of d_head=128 split into two
64-dim sub-heads; dual softmax attention maps combined as A1 - sigmoid(
lambda)*A2, then output projection.

Sharding: core c handles batch b = c//4 and head group hg = c%4 (4 heads).
Host sums the 4 partial output projections per batch.

Key device choices (v2 — fp8 DoubleRow projections + interleaved phases):
  - Q/K/V projections run in fp8e4 DoubleRow mode (two 128-row contraction
    planes per matmul, half engine time).  Precision is recovered with a
    hi/lo residual split of BOTH x and W (3 product chains, lo*lo dropped):
    error ~0.1%, below bf16.  Weights are pre-scaled by 32 on host so fp8
    mantissa stays in the normal range; the 32x on Q,K folds into the exp
    scale (1/8192) and the 32x on V folds into W_o/32 on host.
  - Per-head pipeline: project Q/K for head h, then immediately run its
    attention while head h+1 projects - keeps the scalar engine's exp
    stream running from ~25us instead of idling through a projection
    phase.
  - Scores/PV stay bf16 (fp8 numerics fail the error budget there).
    Scores transposed S^T=[s,t] so softmax'd E^T feeds A@V directly.
  - Softmax denominators: E chunk-pairs folded 3 levels on DVE, then ONE
    gpsimd partition_all_reduce per map gives the per-t sums broadcast
    across partitions - no ones-matmuls, no broadcast matmuls, no PSUM
    footprint for denominators (PSUM budget: sc1 2 + sc2 2 + pv 2 + pj 2
    banks = 8).
  - Softmax without max-subtraction: scores/8 stay within +-10; exp in
    fp32 PSUM -> bf16 is safe.
"""
import sys

sys.path.insert(0, "/opt/trn_rl_repo")

import numpy as np
import ml_dtypes

import concourse.bacc as bacc
import concourse.mybir as mybir
import concourse.tile as tile
from concourse.bass_utils import run_bass_kernel_spmd

# Content-addressed NEFF cache: walrus on this program takes minutes; the
# BIR bytes fully determine the NEFF, so cache across processes.
try:
    import hashlib
    import os as _os
    import pathlib
    import shutil as _sh

    import concourse.bass2jax as _b2j
    import concourse.bass_utils as _bu

    _NEFF_CACHE = pathlib.Path(_os.environ.get("NEFF_CACHE_DIR",
                                               "/tmp/neff_cache"))
    _NEFF_CACHE.mkdir(parents=True, exist_ok=True)
    _orig_cbk = _bu.compile_bir_kernel

    def _cached_cbk(bir_json, tmpdir, neff_name="file.neff"):
        h = hashlib.sha256(bir_json).hexdigest()[:32]
        hit = _NEFF_CACHE / f"{h}_{neff_name}"
        if hit.exists():
            sg = _os.path.join(tmpdir, "sg00")
            _os.makedirs(sg, exist_ok=True)
            dst = _os.path.join(sg, neff_name)
            _sh.copy(hit, dst)
            return dst
        p = _orig_cbk(bir_json, tmpdir, neff_name)
        try:
            _sh.copy(p, hit)
        except OSError:
            pass
        return p

    _bu.compile_bir_kernel = _cached_cbk
    _b2j.compile_bir_kernel = _cached_cbk
except Exception:
    pass

F32 = mybir.dt.float32
BF16 = mybir.dt.bfloat16
F8 = mybir.dt.float8e4
ALU = mybir.AluOpType
EXP = mybir.ActivationFunctionType.Exp
DR = mybir.MatmulPerfMode.DoubleRow

B, T, D, H = 2, 2048, 2048, 16
DH, DS = 128, 64          # head dim, sub-head dim
NCORES = 8
HPC = 4                   # heads per core
HD = HPC * DH             # 512: head-group width
KC = D // 128             # 16 contraction chunks
KP = KC // 2              # 8 DoubleRow chunk-pairs
TG = 4                    # t-groups of 512
SC = T // 128             # 16 s-chunks
WS = 32.0                 # host weight pre-scale (fp8 range)
EXP_SCALE = 1.0 / (8.0 * WS * WS)   # 1/sqrt(DS) / (32*32)

_nc_cache = []
last_result = None  # BassKernelResults of the most recent run (for test.py)


def _build(e_bufs=8, fold=3, fold_bufs=2, use_par=1, ost_eng="scalar",
           pj_bufs=2, pv_bufs=2, sc_bufs=1):
    nc = bacc.Bacc("TRN2", target_bir_lowering=False, debug=False)
    xh_d = nc.dram_tensor("xh", [128, KC, T], F8, kind="ExternalInput")
    xl_d = nc.dram_tensor("xl", [128, KC, T], F8, kind="ExternalInput")
    wq_d = [nc.dram_tensor(f"wq{p}", [128, KC, HD], F8, kind="ExternalInput")
            for p in ("h", "l")]
    wk_d = [nc.dram_tensor(f"wk{p}", [128, KC, HD], F8, kind="ExternalInput")
            for p in ("h", "l")]
    wv_d = [nc.dram_tensor(f"wv{p}", [128, KC, HD], F8, kind="ExternalInput")
            for p in ("h", "l")]
    woT = nc.dram_tensor("woT", [HD, D], BF16, kind="ExternalInput")
    lamb = nc.dram_tensor("lamb", [128, HPC], F32, kind="ExternalInput")
    out = nc.dram_tensor("out", [T, D], BF16, kind="ExternalOutput")

    import concourse.bass_isa as bass_isa
    RED_ADD = bass_isa.ReduceOp.add

    with tile.TileContext(nc) as tc:
        with tc.tile_pool(name="static", bufs=1) as st, \
             tc.tile_pool(name="psum", bufs=1, space="PSUM") as ps:
            xh = st.tile([128, KC, T], F8, name="xh_s")
            xl = st.tile([128, KC, T], F8, name="xl_s")
            vt = [st.tile([128, HD], BF16, name=f"vt{t}") for t in range(SC)]
            qt = [st.tile([128, T], BF16, name=f"qt{i}") for i in range(2)]
            kt = [st.tile([128, T], BF16, name=f"kt{i}") for i in range(2)]
            ho = [st.tile([128, T], BF16, name=f"ho{h}") for h in range(HPC)]
            lamb_sb = st.tile([128, HPC], F32, name="lamb_sb")
            nc.sync.dma_start(lamb_sb[:], lamb[:])

            def load_x():
                # hi planes first: the first two product chains only need
                # xh, so the xl transfer hides behind their matmuls
                for xt, xd in ((xh, xh_d), (xl, xl_d)):
                    for kq in range(4):
                        ksl = slice(4 * kq, 4 * kq + 4)
                        nc.sync.dma_start(xt[:, ksl, :], xd[:, ksl, :])

            def dr_chains(psum_ap, lh, ll, rh, rl):
                """24 DoubleRow matmuls accumulating 3 hi/lo product chains
                (lo*lo dropped) over the full K=2048 contraction.  The
                rl-consuming chain goes last so its operand may still be
                in flight when the group starts."""
                chains = [(lh, rh), (ll, rh), (lh, rl)]
                n = len(chains) * KP
                i = 0
                for (lt, rt) in chains:
                    for kp in range(KP):
                        ksl = slice(2 * kp, 2 * kp + 2)
                        nc.tensor.matmul(psum_ap, lt(ksl), rt(ksl),
                                         start=(i == 0), stop=(i == n - 1),
                                         perf_mode=DR)
                        i += 1

            with tc.tile_pool(name="wqp", bufs=1) as wqp, \
                 tc.tile_pool(name="attn", bufs=1) as at:

                def qkproj_dmas(h):
                    hsl = slice(h * DH, (h + 1) * DH)
                    tiles = []
                    for dr in (wq_d, wk_d):
                        wh = wqp.tile([128, KC, DH], F8, tag="wh", bufs=1,
                                      name="wh")
                        wl = wqp.tile([128, KC, DH], F8, tag="wl", bufs=1,
                                      name="wl")
                        nc.sync.dma_start(wh[:], dr[0][:, :, hsl])
                        nc.sync.dma_start(wl[:], dr[1][:, :, hsl])
                        tiles.append((wh, wl))
                    return tiles

                def qkproj_mms(h, tiles):
                    for dst, (wh, wl) in ((qt[h % 2], tiles[0]),
                                          (kt[h % 2], tiles[1])):
                        for g in range(TG):
                            gsl = slice(g * 512, (g + 1) * 512)
                            pq = ps.tile([128, 512], F32, tag="pj",
                                         bufs=pj_bufs, name="pq")
                            dr_chains(
                                pq[:],
                                lambda k, wh=wh: wh[:, k, :],
                                lambda k, wl=wl: wl[:, k, :],
                                lambda k, gsl=gsl: xh[:, k, gsl],
                                lambda k, gsl=gsl: xl[:, k, gsl])
                            nc.vector.tensor_copy(dst[:, gsl], pq[:])

                def emit_qkproj(h):
                    qkproj_mms(h, qkproj_dmas(h))

                def emit_attn(h, g):
                    hsl = slice(h * DH, (h + 1) * DH)
                    tsl = slice(g * 512, (g + 1) * 512)
                    qh, kh = qt[h % 2], kt[h % 2]
                    e1l, e2l = [], []
                    for sp in range(SC // 2):
                        s1 = ps.tile([128, 1024], F32, tag="sc1",
                                     bufs=sc_bufs, name="s1")
                        s2 = ps.tile([128, 1024], F32, tag="sc2",
                                     bufs=sc_bufs, name="s2")
                        for hf in range(2):
                            ssl = slice((2 * sp + hf) * 128,
                                        (2 * sp + hf + 1) * 128)
                            osl = slice(hf * 512, (hf + 1) * 512)
                            nc.tensor.matmul(s1[:, osl], kh[0:64, ssl],
                                             qh[0:64, tsl],
                                             start=True, stop=True)
                            nc.tensor.matmul(s2[:, osl], kh[64:128, ssl],
                                             qh[64:128, tsl],
                                             start=True, stop=True)
                        e1 = at.tile([128, 1024], BF16, tag="e1",
                                     bufs=e_bufs, name="e1")
                        e2 = at.tile([128, 1024], BF16, tag="e2",
                                     bufs=e_bufs, name="e2")
                        nc.scalar.activation(e1[:], s1[:], EXP,
                                             scale=EXP_SCALE)
                        nc.scalar.activation(e2[:], s2[:], EXP,
                                             scale=EXP_SCALE)
                        e1l.append(e1)
                        e2l.append(e2)

                    # PV accumulation (bf16, K=128 per s-chunk)
                    p1 = ps.tile([128, 512], F32, tag="pv", bufs=pv_bufs,
                                 name="p1")
                    p2 = ps.tile([128, 512], F32, tag="pv", bufs=pv_bufs,
                                 name="p2")
                    for sp in range(SC // 2):
                        for hf in range(2):
                            s = 2 * sp + hf
                            osl = slice(hf * 512, (hf + 1) * 512)
                            st_, sp_ = (s == 0), (s == SC - 1)
                            nc.tensor.matmul(p1[:], vt[s][:, hsl],
                                             e1l[sp][:, osl],
                                             start=st_, stop=sp_)
                            nc.tensor.matmul(p2[:], vt[s][:, hsl],
                                             e2l[sp][:, osl],
                                             start=st_, stop=sp_)

                    # fold E chunk-pairs for the denominator reduction
                    f1l, f2l = e1l, e2l
                    for lvl in range(fold):
                        n = len(f1l) // 2
                        if n == 0:
                            break
                        nf1, nf2 = [], []
                        for j in range(n):
                            f1 = at.tile([128, 1024], BF16, tag=f"f1_{lvl}",
                                         bufs=fold_bufs, name="f1")
                            f2 = at.tile([128, 1024], BF16, tag=f"f2_{lvl}",
                                         bufs=fold_bufs, name="f2")
                            # adjacent pairing so fold_bufs=2 can't
                            # slot-deadlock (consumer of slot k only needs
                            # already-written tiles)
                            nc.vector.tensor_add(f1[:], f1l[2 * j][:],
                                                 f1l[2 * j + 1][:])
                            nc.vector.tensor_add(f2[:], f2l[2 * j][:],
                                                 f2l[2 * j + 1][:])
                            nf1.append(f1)
                            nf2.append(f2)
                        f1l, f2l = nf1, nf2

                    # denominators: partition reduce-broadcast on gpsimd,
                    # then fold the two t-halves and invert on DVE
                    rs1 = at.tile([128, 1024], BF16, tag="rs", bufs=2,
                                  name="rs1")
                    rs2 = at.tile([128, 1024], BF16, tag="rs", bufs=2,
                                  name="rs2")
                    nc.gpsimd.partition_all_reduce(rs1[:], f1l[0][:],
                                                   channels=128,
                                                   reduce_op=RED_ADD)
                    nc.gpsimd.partition_all_reduce(rs2[:], f2l[0][:],
                                                   channels=128,
                                                   reduce_op=RED_ADD)
                    ha1 = at.tile([128, 512], BF16, tag="ha", bufs=2,
                                  name="ha1")
                    ha2 = at.tile([128, 512], BF16, tag="ha", bufs=2,
                                  name="ha2")
                    nc.vector.tensor_add(ha1[:], rs1[:, 0:512],
                                         rs1[:, 512:1024])
                    nc.vector.tensor_add(ha2[:], rs2[:, 0:512],
                                         rs2[:, 512:1024])
                    rb1 = at.tile([128, 512], BF16, tag="rb1", bufs=2,
                                  name="rb1")
                    rc2 = at.tile([128, 512], BF16, tag="rc2", bufs=2,
                                  name="rc2")
                    rb2 = at.tile([128, 512], BF16, tag="rb2", bufs=2,
                                  name="rb2")
                    with nc.allow_low_precision(reason="softmax denom"):
                        nc.vector.reciprocal(rb1[:], ha1[:])
                        nc.vector.reciprocal(rc2[:], ha2[:])
                    nc.vector.tensor_scalar(
                        rb2[:], rc2[:], lamb_sb[:, h:h + 1], None, ALU.mult)

                    tm1 = at.tile([128, 512], BF16, tag="tm1", bufs=2,
                                  name="tm1")
                    tm2 = at.tile([128, 512], BF16, tag="tm2", bufs=2,
                                  name="tm2")
                    nc.vector.tensor_mul(tm1[:], p1[:], rb1[:])
                    nc.vector.tensor_mul(tm2[:], p2[:], rb2[:])
                    nc.vector.tensor_sub(ho[h][:, tsl], tm1[:], tm2[:])

                t0 = qkproj_dmas(0)
                load_x()
                qkproj_mms(0, t0)
                # V projection: scoped so its 16KB of fp8 weights free early
                with tc.tile_pool(name="wvp", bufs=1) as wvp:
                    wvh = wvp.tile([128, KC, HD], F8, name="wvh_s")
                    wvl = wvp.tile([128, KC, HD], F8, name="wvl_s")
                    nc.sync.dma_start(wvh[:], wv_d[0][:])
                    nc.sync.dma_start(wvl[:], wv_d[1][:])
                    for t in range(SC):
                        pvv = ps.tile([128, HD], F32, tag="pj",
                                      bufs=pj_bufs, name="pvv")
                        tsl = slice(t * 128, (t + 1) * 128)
                        dr_chains(
                            pvv[:],
                            lambda k, tsl=tsl: xh[:, k, tsl],
                            lambda k, tsl=tsl: xl[:, k, tsl],
                            lambda k: wvh[:, k, :],
                            lambda k: wvl[:, k, :])
                        nc.vector.tensor_copy(vt[t][:], pvv[:])

                # ---------------- output projection ----------------
                # wo loads early (reusing the freed V-weight SBUF); oproj
                # t-chunks emit as soon as head 3 finishes their t-range so
                # the projection overlaps the last head instead of tailing.
                with tc.tile_pool(name="oproj", bufs=1) as op:
                    wo = []
                    for c in range(HPC):
                        woc = op.tile([128, T], BF16, name=f"wo{c}")
                        nc.sync.dma_start(woc[:],
                                          woT[c * 128:(c + 1) * 128, :])
                        wo.append(woc)
                    otags = ["sc1", "sc2", "pv", "pj"]

                    def emit_oproj(trange):
                        for t in trange:
                            pol = [ps.tile([128, 512], F32, tag=otags[mg],
                                           bufs=(sc_bufs if mg < 2 else
                                                 (pv_bufs if mg == 2 else
                                                  pj_bufs)),
                                           name="po")
                                   for mg in range(TG)]
                            for c in range(HPC):
                                for mg in range(TG):
                                    nc.tensor.matmul(
                                        pol[mg][:],
                                        ho[c][:, t * 128:(t + 1) * 128],
                                        wo[c][:, mg * 512:(mg + 1) * 512],
                                        start=(c == 0), stop=(c == HPC - 1))
                            for mg in range(TG):
                                ost = op.tile([128, 512], BF16, tag="ost",
                                              bufs=3, name="ost")
                                if ost_eng == "scalar":
                                    nc.scalar.copy(ost[:], pol[mg][:])
                                else:
                                    nc.vector.tensor_copy(ost[:], pol[mg][:])
                                nc.sync.dma_start(
                                    out[t * 128:(t + 1) * 128,
                                        mg * 512:(mg + 1) * 512], ost[:])

                    for h in range(HPC):
                        if h > 0:
                            emit_qkproj(h)
                        for g in range(TG):
                            emit_attn(h, g)
                    emit_oproj(range(SC))

    nc.compile()
    return nc


def _prep_inputs(x, W_q, W_k, W_v, W_o, lambda_param):
    f8 = ml_dtypes.float8_e4m3fn
    bf = ml_dtypes.bfloat16
    lam = 1.0 / (1.0 + np.exp(-lambda_param))  # sigmoid, [H]

    def kmajor(a2d, width):
        # [D, width] -> [128, KC, width]
        return np.ascontiguousarray(
            a2d.reshape(KC, 128, width).transpose(1, 0, 2))

    def hilo(a):
        hi = a.astype(f8)
        lo = (a - hi.astype(np.float32)).astype(f8)
        return hi, lo

    in_maps = []
    for c in range(NCORES):
        b, hg = c // HPC, c % HPC
        hs = hg * HD
        xT = kmajor(np.ascontiguousarray(x[b].T), T)
        xh, xl = hilo(xT)
        m = {"xh": xh, "xl": xl}
        for nm, W in (("wq", W_q), ("wk", W_k), ("wv", W_v)):
            wT = kmajor(np.ascontiguousarray(W[hs:hs + HD, :].T) * WS, HD)
            m[nm + "h"], m[nm + "l"] = hilo(wT)
        m["woT"] = (np.ascontiguousarray(W_o[:, hs:hs + HD].T)
                    / WS).astype(bf)
        m["lamb"] = np.broadcast_to(
            lam[hg * HPC:(hg + 1) * HPC][None, :], (128, HPC)
        ).astype(np.float32).copy()
        in_maps.append(m)
    return in_maps


def kernel(x, W_q, W_k, W_v, W_o, lambda_param):
    x = np.asarray(x, dtype=np.float32)
    W_q = np.asarray(W_q, dtype=np.float32)
    W_k = np.asarray(W_k, dtype=np.float32)
    W_v = np.asarray(W_v, dtype=np.float32)
    W_o = np.asarray(W_o, dtype=np.float32)
    lambda_param = np.asarray(lambda_param, dtype=np.float32)

    in_maps = _prep_inputs(x, W_q, W_k, W_v, W_o, lambda_param)

    if not _nc_cache:
        _nc_cache.append(_build())
    nc = _nc_cache[0]

    res = run_bass_kernel_spmd(nc, in_maps, core_ids=list(range(NCORES)))
    global last_result
    last_result = res
    outp = np.zeros((B, T, D), dtype=np.float32)
    for c in range(NCORES):
        outp[c // HPC] += res.results[c]["out"].astype(np.float32)
    return outp



# revision 3
# speedup vs baseline: 1.0331x; 1.0331x over previous
"""Differential attention kernel for 8 Trainium2 NeuronCores — v3.

Problem: B=2, T=2048, D=2048, H=16 heads of d_head=128 split into two
64-dim sub-heads; dual softmax maps combined as A1 - sigmoid(lambda)*A2,
then output projection.

Sharding: core c handles batch b = c//4 and head group hg = c%4 (4 heads).
Host sums the 4 partial output projections per batch.

v3 over v2 (479us): everything that can be fp8 DoubleRow is, and emission
is a software pipeline so the PE never head-of-line blocks on the
activation engine:
  - Scores in fp8 DR: per head/map, Q and K split hi/lo on device into
    packs (lhsT planes [Kh;Kh],[Kl;Kl]; rhs [Qh;Ql] broadcast to both
    planes) -> exact (Kh+Kl)^T(Qh+Ql), one DR matmul per s-chunk instead
    of one bf16 matmul: half the PE cycles, ~9e-4 rel.
  - O-projection in fp8 DR, 3 hi/lo chains; W_o pre-scaled x32 on host
    for fp8 range, 1/1024 descale folded into the DVE psum->sbuf copy.
  - Interleaved emission: per (head, tgroup) window the sp-loop emits
    [scores sp | PV sp-1 | fold | filler] where filler drains a queue of
    V-proj / next-head-QK-proj / O-proj units. Scores serialize behind
    exp reads (sc psum bufs=1) so filler keeps the PE busy in the gaps.
  - Engine rebalance: fold level-1 + pack dups on gpsimd, K-pack hi
    copies on scalar, ost copies on DVE.
PSUM: sc1 2 + sc2 2 + pv 2 + pj 2 banks = 8; V/QK/O-proj share pj.
"""
import sys

sys.path.insert(0, "/opt/trn_rl_repo")

import numpy as np
import ml_dtypes

import concourse.bacc as bacc
import concourse.mybir as mybir
import concourse.tile as tile
from concourse.bass_utils import run_bass_kernel_spmd

# Content-addressed NEFF cache: walrus on this program takes minutes; the
# BIR bytes fully determine the NEFF, so cache across processes.
try:
    import hashlib
    import os as _os
    import pathlib
    import shutil as _sh

    import concourse.bass2jax as _b2j
    import concourse.bass_utils as _bu

    _NEFF_CACHE = pathlib.Path(_os.environ.get("NEFF_CACHE_DIR",
                                               "/tmp/neff_cache"))
    _NEFF_CACHE.mkdir(parents=True, exist_ok=True)
    _orig_cbk = _bu.compile_bir_kernel

    def _cached_cbk(bir_json, tmpdir, neff_name="file.neff"):
        h = hashlib.sha256(bir_json).hexdigest()[:32]
        hit = _NEFF_CACHE / f"{h}_{neff_name}"
        if hit.exists():
            sg = _os.path.join(tmpdir, "sg00")
            _os.makedirs(sg, exist_ok=True)
            dst = _os.path.join(sg, neff_name)
            _sh.copy(hit, dst)
            return dst
        p = _orig_cbk(bir_json, tmpdir, neff_name)
        try:
            _sh.copy(p, hit)
        except OSError:
            pass
        return p

    _bu.compile_bir_kernel = _cached_cbk
    _b2j.compile_bir_kernel = _cached_cbk
except Exception:
    pass

F32 = mybir.dt.float32
BF16 = mybir.dt.bfloat16
F8 = mybir.dt.float8e4
ALU = mybir.AluOpType
EXP = mybir.ActivationFunctionType.Exp
DR = mybir.MatmulPerfMode.DoubleRow

B, T, D, H = 2, 2048, 2048, 16
DH, DS = 128, 64          # head dim, sub-head dim
NCORES = 8
HPC = 4                   # heads per core
HD = HPC * DH             # 512: head-group width
KC = D // 128             # 16 contraction chunks
KP = KC // 2              # 8 DoubleRow chunk-pairs
TG = 4                    # t-groups of 512
SC = T // 128             # 16 s-chunks
WS = 32.0                 # host weight pre-scale (fp8 range)
WOS = 32.0                # W_o pre-scale; ho carries x32 from V
OSC = 1.0 / (WS * WOS)    # oproj psum descale in the ost copy
EXP_SCALE = 1.0 / (8.0 * WS * WS)   # 1/sqrt(DS) / (32*32)

_nc_cache = []
last_result = None  # BassKernelResults of the most recent run (for test.py)


def _build():
    nc = bacc.Bacc("TRN2", target_bir_lowering=False, debug=False)
    xh_d = nc.dram_tensor("xh", [128, KC, T], F8, kind="ExternalInput")
    xl_d = nc.dram_tensor("xl", [128, KC, T], F8, kind="ExternalInput")
    wq_d = [nc.dram_tensor(f"wq{p}", [128, KC, HD], F8, kind="ExternalInput")
            for p in ("h", "l")]
    wk_d = [nc.dram_tensor(f"wk{p}", [128, KC, HD], F8, kind="ExternalInput")
            for p in ("h", "l")]
    wv_d = [nc.dram_tensor(f"wv{p}", [128, KC, HD], F8, kind="ExternalInput")
            for p in ("h", "l")]
    wo_d = [nc.dram_tensor(f"wo{p}", [128, HPC, D], F8, kind="ExternalInput")
            for p in ("h", "l")]
    lamb = nc.dram_tensor("lamb", [128, HPC], F32, kind="ExternalInput")
    out = nc.dram_tensor("out", [T, D], BF16, kind="ExternalOutput")

    import concourse.bass_isa as bass_isa
    RED_ADD = bass_isa.ReduceOp.add

    with tile.TileContext(nc) as tc:
        with tc.tile_pool(name="static", bufs=1) as st, \
             tc.tile_pool(name="psum", bufs=1, space="PSUM") as ps:
            xh = st.tile([128, KC, T], F8, name="xh_s")
            xl = st.tile([128, KC, T], F8, name="xl_s")
            vt = st.tile([128, SC, HD], BF16, name="vt_s")
            # per-parity fp8 score packs: q [Qh;Ql], k planes [Kh;Kh],[Kl;Kl]
            qp = [[st.tile([128, T], F8, name=f"qp{i}_{m}") for m in range(2)]
                  for i in range(2)]
            kp = [[st.tile([128, 2, T], F8, name=f"kp{i}_{m}")
                   for m in range(2)] for i in range(2)]
            hoh = st.tile([128, HPC, T], F8, name="hoh_s")
            hol = st.tile([128, HPC, T], F8, name="hol_s")
            lamb_sb = st.tile([128, HPC], F32, name="lamb_sb")
            nc.sync.dma_start(lamb_sb[:], lamb[:])

            def load_x_g(g, pieces=1):
                gsl = slice(g * 512, (g + 1) * 512)
                for xt, xd in ((xh, xh_d), (xl, xl_d)):
                    for p in range(pieces):
                        ksl = slice(p * KC // pieces,
                                    (p + 1) * KC // pieces)
                        nc.sync.dma_start(xt[:, ksl, gsl],
                                          xd[:, ksl, gsl])

            def dr_chains(psum_ap, lh, ll, rh, rl):
                """24 DoubleRow matmuls accumulating 3 hi/lo product chains
                (lo*lo dropped) over the full K=2048 contraction."""
                chains = [(lh, rh), (ll, rh), (lh, rl)]
                n = len(chains) * KP
                i = 0
                for (lt, rt) in chains:
                    for kpr in range(KP):
                        ksl = slice(2 * kpr, 2 * kpr + 2)
                        nc.tensor.matmul(psum_ap, lt(ksl), rt(ksl),
                                         start=(i == 0), stop=(i == n - 1),
                                         perf_mode=DR)
                        i += 1

            with tc.tile_pool(name="wqp", bufs=1) as wqp, \
                 tc.tile_pool(name="attn", bufs=1) as at:

                def qkproj_dmas(h):
                    hsl = slice(h * DH, (h + 1) * DH)
                    tl = {}
                    # K weights first: K proj gates the first scores
                    for nm, dr in (("k", wk_d), ("q", wq_d)):
                        wh = wqp.tile([128, KC, DH], F8, tag=f"w{nm}h",
                                      bufs=2, name=f"w{nm}h")
                        wl = wqp.tile([128, KC, DH], F8, tag=f"w{nm}l",
                                      bufs=2, name=f"w{nm}l")
                        nc.sync.dma_start(wh[:], dr[0][:, :, hsl])
                        nc.sync.dma_start(wl[:], dr[1][:, :, hsl])
                        tl[nm] = (wh, wl)
                    return [tl["q"], tl["k"]]

                def qproj_unit(h, g, tiles):
                    """Q proj tile + fp8 hi/lo pack split for both maps."""
                    wh, wl = tiles[0]
                    gsl = slice(g * 512, (g + 1) * 512)
                    pq = ps.tile([128, 512], F32, tag="pj", bufs=2,
                                 name="pq")
                    dr_chains(pq[:],
                              lambda k: wh[:, k, :], lambda k: wl[:, k, :],
                              lambda k: xh[:, k, gsl],
                              lambda k: xl[:, k, gsl])
                    q1, q2 = qp[h % 2]
                    with nc.allow_low_precision(reason="fp8 pack"):
                        # map1: rows 0:64 of psum
                        nc.vector.tensor_copy(q1[0:64, gsl], pq[0:64, :])
                        nc.vector.tensor_sub(q1[64:128, gsl], pq[0:64, :],
                                             q1[0:64, gsl])
                        # map2: rows 64:128 of psum, shifted down
                        nc.vector.tensor_copy(q2[0:64, gsl], pq[64:128, :])
                        nc.vector.tensor_sub(q2[64:128, gsl], pq[64:128, :],
                                             q2[0:64, gsl])

                def kproj_unit(h, g, tiles):
                    """K proj tile + duplicated-plane fp8 packs."""
                    wh, wl = tiles[1]
                    gsl = slice(g * 512, (g + 1) * 512)
                    pk = ps.tile([128, 512], F32, tag="pj", bufs=2,
                                 name="pk")
                    dr_chains(pk[:],
                              lambda k: wh[:, k, :], lambda k: wl[:, k, :],
                              lambda k: xh[:, k, gsl],
                              lambda k: xl[:, k, gsl])
                    k1, k2 = kp[h % 2]
                    with nc.allow_low_precision(reason="fp8 pack"):
                        # map1: hi, dup up, lo, dup up
                        nc.vector.tensor_copy(k1[0:64, 0, gsl],
                                              pk[0:64, :])
                        nc.gpsimd.tensor_copy(k1[64:128, 0, gsl],
                                              k1[0:64, 0, gsl])
                        nc.vector.tensor_sub(k1[0:64, 1, gsl], pk[0:64, :],
                                             k1[0:64, 0, gsl])
                        nc.gpsimd.tensor_copy(k1[64:128, 1, gsl],
                                              k1[0:64, 1, gsl])
                        # map2: hi shifted down, dup up, lo aligned at 64,
                        # dup down
                        nc.vector.tensor_copy(k2[0:64, 0, gsl],
                                              pk[64:128, :])
                        nc.gpsimd.tensor_copy(k2[64:128, 0, gsl],
                                              k2[0:64, 0, gsl])
                        nc.vector.tensor_sub(k2[64:128, 1, gsl],
                                             pk[64:128, :],
                                             k2[64:128, 0, gsl])
                        nc.gpsimd.tensor_copy(k2[0:64, 1, gsl],
                                              k2[64:128, 1, gsl])

                # ---- window: one (head, tgroup) of scores/exp/PV with
                # filler units drained into the PE stall slots.  Each unit
                # has a due-slot; emission order == PE execution order, so
                # spreading units over their host windows keeps the PE fed
                # exactly where the act-paced scores pipeline would stall.
                import bisect

                filler = []          # sorted list of (due_slot, seq, fn)
                fseq = [0]
                cur_slot = [0]

                def push_filler(due, fn):
                    bisect.insort(filler, (due, fseq[0], fn))
                    fseq[0] += 1

                def pop_fillers():
                    while filler and filler[0][0] <= cur_slot[0]:
                        filler.pop(0)[2]()
                    cur_slot[0] += 1

                def emit_scores_s(h, g, s, el):
                    tsl = slice(g * 512, (g + 1) * 512)
                    k1, k2 = kp[h % 2]
                    q1, q2 = qp[h % 2]
                    r1 = q1[:, tsl].unsqueeze(1).broadcast_to([128, 2, 512])
                    r2 = q2[:, tsl].unsqueeze(1).broadcast_to([128, 2, 512])
                    # both maps' scores for one s-chunk in a 2-bank psum
                    # tile, double-buffered so scores(s+1) overlaps exp(s)
                    sB = ps.tile([128, 1024], F32, tag="sc", bufs=2,
                                 name="sB")
                    ssl = slice(s * 128, (s + 1) * 128)
                    nc.tensor.matmul(sB[:, 0:512], k1[:, :, ssl], r1,
                                     start=True, stop=True, perf_mode=DR)
                    nc.tensor.matmul(sB[:, 512:1024], k2[:, :, ssl], r2,
                                     start=True, stop=True, perf_mode=DR)
                    e = at.tile([128, 1024], BF16, tag="e", bufs=5,
                                name="e")
                    nc.scalar.activation(e[:], sB[:], EXP, scale=EXP_SCALE)
                    el.append(e)

                def emit_pv_s(h, s, el, p1, p2):
                    hsl = slice(h * DH, (h + 1) * DH)
                    st_, sp_ = (s == 0), (s == SC - 1)
                    nc.tensor.matmul(p1[:], vt[:, s, hsl],
                                     el[s][:, 0:512],
                                     start=st_, stop=sp_)
                    nc.tensor.matmul(p2[:], vt[:, s, hsl],
                                     el[s][:, 512:1024],
                                     start=st_, stop=sp_)

                def emit_fold(lvl, j, srcl, dstl):
                    # pair-add of level inputs (2j, 2j+1) on DVE
                    f1 = at.tile([128, 1024], BF16, tag=f"f_{lvl}", bufs=2,
                                 name="f")
                    nc.vector.tensor_add(f1[:], srcl[2 * j][:],
                                         srcl[2 * j + 1][:])
                    dstl.append(f1)

                def make_denoms_chain(h, g, g2l, pc1, pc2, rush=False):
                    """The lazy denominator+ho chain for window (h, g) as
                    pieces dripped into the NEXT window's slots, so its
                    cross-engine hops never sit ahead of urgent pack/dup
                    work in the DVE/Pool queues."""
                    tsl = slice(g * 512, (g + 1) * 512)
                    fin1 = at.tile([128, 1024], BF16, tag="fin", bufs=1,
                                   name="fin1")
                    rs1 = at.tile([128, 1024], BF16, tag="rs", bufs=1,
                                  name="rs1")
                    rb1 = at.tile([128, 512], BF16, tag="rb1", bufs=2,
                                  name="rb1")
                    rc2 = at.tile([128, 512], BF16, tag="rc2", bufs=2,
                                  name="rc2")
                    rb2 = at.tile([128, 512], BF16, tag="rb2", bufs=2,
                                  name="rb2")
                    tm1 = at.tile([128, 512], BF16, tag="tm1", bufs=2,
                                  name="tm1")
                    tm2 = at.tile([128, 512], BF16, tag="tm2", bufs=2,
                                  name="tm2")
                    dt_ = at.tile([128, 512], BF16, tag="dtmp", bufs=2,
                                  name="dt")

                    def p_fin():
                        nc.vector.tensor_add(fin1[:], g2l[0][:], g2l[1][:])

                    def p_rs():
                        nc.gpsimd.partition_all_reduce(rs1[:], fin1[:],
                                                       channels=128,
                                                       reduce_op=RED_ADD)

                    def p_recip():
                        with nc.allow_low_precision(reason="denom"):
                            nc.vector.reciprocal(rb1[:], rs1[:, 0:512])
                            nc.vector.reciprocal(rc2[:], rs1[:, 512:1024])
                        nc.vector.tensor_scalar(
                            rb2[:], rc2[:], lamb_sb[:, h:h + 1], None,
                            ALU.mult)

                    eng = nc.vector if rush else nc.gpsimd

                    def p_tm():
                        eng.tensor_mul(tm1[:], pc1[:], rb1[:])
                        eng.tensor_mul(tm2[:], pc2[:], rb2[:])




                    def p_dt():
                        eng.tensor_sub(dt_[:], tm1[:], tm2[:])

                    def p_ho():
                        with nc.allow_low_precision(reason="ho fp8"):
                            eng.tensor_copy(hoh[:, h, tsl], dt_[:])
                            eng.tensor_sub(hol[:, h, tsl], dt_[:],
                                           hoh[:, h, tsl])

                    return [p_fin, p_rs, p_recip, p_tm, p_dt, p_ho]

                pending_chain = [[]]
                pending_pv = [None]

                def finish_window(rush=False):
                    """Last PV + psum drain of the previous window; run at
                    slot 0 of the next window so the PE never waits on the
                    final exp at a window boundary.  For the very last
                    window (rush) the drain is skipped: no later window
                    needs the pv banks, so tm reads the psum directly."""
                    if pending_pv[0] is None:
                        return
                    h, el, p1, p2, g2l, g = pending_pv[0]
                    pending_pv[0] = None
                    emit_pv_s(h, SC - 1, el, p1, p2)
                    pc1 = at.tile([128, 512], BF16, tag="pc1", bufs=2,
                                  name="pc1")
                    pc2 = at.tile([128, 512], BF16, tag="pc2", bufs=2,
                                  name="pc2")
                    nc.vector.tensor_copy(pc1[:], p1[:])
                    nc.vector.tensor_copy(pc2[:], p2[:])
                    pending_chain[0] = make_denoms_chain(h, g, g2l,
                                                         pc1, pc2,
                                                         rush=rush)

                def window(h, g):
                    el, f0l, g1l, g2l = [], [], [], []
                    p1 = ps.tile([128, 512], F32, tag="pv", bufs=2,
                                 name="p1")
                    p2 = ps.tile([128, 512], F32, tag="pv", bufs=2,
                                 name="p2")
                    for s in range(SC):
                        emit_scores_s(h, g, s, el)
                        if s == 0:
                            finish_window()
                        if s > 0:
                            emit_pv_s(h, s - 1, el, p1, p2)
                        if s % 2 == 1:
                            emit_fold(0, s // 2, el, f0l)
                        if s % 4 == 3:
                            emit_fold(1, s // 4, f0l, g1l)
                        if s % 8 == 7:
                            emit_fold(2, s // 8, g1l, g2l)
                        if s >= 2 and pending_chain[0]:
                            pending_chain[0].pop(0)()
                        pop_fillers()
                    pending_pv[0] = (h, el, p1, p2, g2l, g)

                # ----------------- emission schedule -----------------
                # startup: K h0 weights, then x tgroup 0 in k-chunk
                # pieces (first K-proj matmuls start after ~256KB), then
                # Q weights and the rest of x
                hsl0 = slice(0, DH)
                t0k = [wqp.tile([128, KC, DH], F8, tag=f"wk{p}", bufs=2,
                                name=f"wk{p}") for p in "hl"]
                t0q = [wqp.tile([128, KC, DH], F8, tag=f"wq{p}", bufs=2,
                                name=f"wq{p}") for p in "hl"]
                nc.sync.dma_start(t0k[0][:], wk_d[0][:, :, hsl0])
                nc.sync.dma_start(t0k[1][:], wk_d[1][:, :, hsl0])
                load_x_g(0, pieces=4)
                nc.sync.dma_start(t0q[0][:], wq_d[0][:, :, hsl0])
                nc.sync.dma_start(t0q[1][:], wq_d[1][:, :, hsl0])
                load_x_g(1, pieces=2)
                t0 = [tuple(t0q), tuple(t0k)]

                def vproj_unit(t, wvh, wvl):
                    pvv = ps.tile([128, HD], F32, tag="pj", bufs=2,
                                  name="pvv")
                    tsl = slice(t * 128, (t + 1) * 128)
                    dr_chains(pvv[:],
                              lambda k: xh[:, k, tsl],
                              lambda k: xl[:, k, tsl],
                              lambda k: wvh[:, k, :],
                              lambda k: wvl[:, k, :])
                    nc.vector.tensor_copy(vt[:, t, :], pvv[:])

                with tc.tile_pool(name="wvp", bufs=1) as wvp:
                    wvh = wvp.tile([128, KC, HD], F8, name="wvh_s")
                    wvl = wvp.tile([128, KC, HD], F8, name="wvl_s")
                    nc.sync.dma_start(wvh[:], wv_d[0][:])
                    nc.sync.dma_start(wvl[:], wv_d[1][:])
                    for g in range(2, TG):
                        load_x_g(g, pieces=2)

                    # up front: what window (0,0) needs - the full K
                    # pack of head 0, Q tgroup 0, first V chunks
                    for g in range(TG):
                        kproj_unit(0, g, t0)
                    qproj_unit(0, 0, t0)
                    for t in range(4):
                        vproj_unit(t, wvh, wvl)
                    # PV(0,0,s) at slot s+1 needs vt chunk s: unit t due
                    # by slot t (4 emitted up front)
                    for t in range(4, SC):
                        push_filler(max(0, t - 3),
                                    lambda t=t: vproj_unit(t, wvh, wvl))
                    # Q pack is per-tgroup: Q(0,g) due a window early
                    for g in range(1, TG):
                        push_filler(16 * (g - 1) + 4,
                                    lambda g=g: qproj_unit(0, g, t0))

                    def push_qkproj(h):
                        # K of head h spread over head h-1's windows
                        # (scores need the full K pack at (h,0)); Q units
                        # slide later, each due one window before use
                        th = qkproj_dmas(h)
                        base = 64 * (h - 1)
                        for g in range(TG):
                            push_filler(base + 8 * g,
                                        lambda g=g, th=th: kproj_unit(
                                            h, g, th))
                        for g in range(TG):
                            due = (base + 40 if g == 0
                                   else 64 * h + 16 * (g - 1) + 4)
                            push_filler(due,
                                        lambda g=g, th=th: qproj_unit(
                                            h, g, th))

                    push_qkproj(1)
                    for g in range(TG):
                        window(0, g)
                    # all vproj units (due <= 7) drained inside window
                    # (0, 0); wvp is safe to close

                def oproj_mg(t, mg, tail=False):
                    """One [128 t x 512] slice of the output projection:
                    6 DR matmuls + descale copy + dma."""
                    tsl = slice(t * 128, (t + 1) * 128)
                    msl = slice(mg * 512, (mg + 1) * 512)
                    if tail and (t * TG + mg) % 2 == 0:
                        # windows are done: borrow the idle score psum
                        # banks for 4-deep pol pipelining
                        pol = ps.tile([128, 1024], F32, tag="sc",
                                      bufs=2, name="polt")[:, 0:512]
                    else:
                        pol = ps.tile([128, 512], F32, tag="pj",
                                      bufs=2, name="pol")[:]
                    # heads (0,1) planes first: they are ready a whole
                    # head-phase before heads (2,3), so the psum group
                    # starts without waiting on the last ho chain
                    i = 0
                    for c in (0, 2):
                        for lt, rt in ((hoh, wosb[0]), (hol, wosb[0]),
                                       (hoh, wosb[1])):
                            nc.tensor.matmul(
                                pol, lt[:, c:c + 2, tsl],
                                rt[:, c:c + 2, msl],
                                start=(i == 0), stop=(i == 5),
                                perf_mode=DR)
                            i += 1
                    ost = at.tile([128, 512], BF16, tag="ost", bufs=6,
                                  name="ost")
                    with nc.allow_low_precision(reason="oproj descale"):
                        nc.vector.tensor_scalar(ost[:], pol, OSC,
                                                None, ALU.mult)
                    nc.sync.dma_start(out[tsl, msl], ost[:])

                with tc.tile_pool(name="wop", bufs=1) as wop:
                    wosb = []
                    for p in range(2):
                        w8 = wop.tile([128, HPC, D], F8, name=f"wo8{p}")
                        nc.sync.dma_start(w8[:], wo_d[p][:])
                        wosb.append(w8)

                    for h in range(1, HPC):
                        if h < HPC - 1:
                            push_qkproj(h + 1)
                        for g in range(TG):
                            window(h, g)
                            if h == HPC - 1:
                                # ho(3,g) lands ~slot 6 of window (3,g+1)
                                # via the deferred chain; oproj mg-chains
                                # for tgroup g follow, 2 per slot
                                tail = g == TG - 1
                                for i in range(4 * TG):
                                    t = 4 * g + i // TG
                                    mg = i % TG
                                    push_filler(
                                        192 + 16 * (g + 1) + 8 + i // 4,
                                        lambda t=t, mg=mg, tl=tail:
                                        oproj_mg(t, mg, tail=tl))
                    # drain: last PV + its chain (PE still has tgroup-2
                    # oproj in flight), then the tgroup-3 oproj units
                    finish_window(rush=True)
                    for p in pending_chain[0]:
                        p()
                    pending_chain[0] = []
                    while filler:
                        filler.pop(0)[2]()

    nc.compile()
    return nc


def _prep_inputs(x, W_q, W_k, W_v, W_o, lambda_param):
    f8 = ml_dtypes.float8_e4m3fn
    lam = 1.0 / (1.0 + np.exp(-lambda_param))  # sigmoid, [H]

    def kmajor(a2d, width):
        # [D, width] -> [128, KC, width]
        return np.ascontiguousarray(
            a2d.reshape(KC, 128, width).transpose(1, 0, 2))

    def hilo(a):
        hi = a.astype(f8)
        lo = (a - hi.astype(np.float32)).astype(f8)
        return hi, lo

    in_maps = []
    for c in range(NCORES):
        b, hg = c // HPC, c % HPC
        hs = hg * HD
        xT = kmajor(np.ascontiguousarray(x[b].T), T)
        xh, xl = hilo(xT)
        m = {"xh": xh, "xl": xl}
        for nm, W in (("wq", W_q), ("wk", W_k), ("wv", W_v)):
            wT = kmajor(np.ascontiguousarray(W[hs:hs + HD, :].T) * WS, HD)
            m[nm + "h"], m[nm + "l"] = hilo(wT)
        # W_o slice [D, 512] -> [512, D] -> [128, 4, D], x32 for fp8 range
        woT = np.ascontiguousarray(W_o[:, hs:hs + HD].T) * WOS
        wo8 = np.ascontiguousarray(
            woT.reshape(HPC, 128, D).transpose(1, 0, 2))
        m["woh"], m["wol"] = hilo(wo8)
        m["lamb"] = np.broadcast_to(
            lam[hg * HPC:(hg + 1) * HPC][None, :], (128, HPC)
        ).astype(np.float32).copy()
        in_maps.append(m)
    return in_maps


def kernel(x, W_q, W_k, W_v, W_o, lambda_param):
    x = np.asarray(x, dtype=np.float32)
    W_q = np.asarray(W_q, dtype=np.float32)
    W_k = np.asarray(W_k, dtype=np.float32)
    W_v = np.asarray(W_v, dtype=np.float32)
    W_o = np.asarray(W_o, dtype=np.float32)
    lambda_param = np.asarray(lambda_param, dtype=np.float32)

    in_maps = _prep_inputs(x, W_q, W_k, W_v, W_o, lambda_param)

    if not _nc_cache:
        _nc_cache.append(_build())
    nc = _nc_cache[0]

    res = run_bass_kernel_spmd(nc, in_maps, core_ids=list(range(NCORES)))
    global last_result
    last_result = res
    outp = np.zeros((B, T, D), dtype=np.float32)
    for c in range(NCORES):
        outp[c // HPC] += res.results[c]["out"].astype(np.float32)
    return outp
